# Initial kernel scaffold
#
"""Trainium2 Bass kernel for CapsuleLayer dynamic routing.

Problem: u = einsum('bpe,pjed->bpjd', inp, W[0]) + b, then 3 routing
iterations (softmax over j, weighted sum over p, squash) -> vj [B,J,D].

Shapes: B=16, P=1024, J=32, Dp=D=64.  W is 512MB fp32 -> DMA dominated.

Strategy (8 NeuronCores):
 - Shard P across cores: 128 p's per core; all batches on every core.
 - Host packs W (bf16) as stacked pairs: rhs[pair] = [W_pe; W_po] with
   K=128=(2 p's x 64 e).  lhsT[pair] = block-diag([inp_pe, inp_po]) so one
   matmul computes u for 2 capsules with full contraction rows, M=32.
 - 4 pairs per PSUM round via col-group tile_position -> PSUM [128,1024]
   halves, double buffered -> wide evictions (cast bf16) into SBUF
   u[(k,b), (g,d,j)].  Free order (d,j) keeps every routing DVE op in the
   bf16 2x mode: v/c broadcasts are outer/middle dims and the d-reduction
   is a pairwise tree of step-1 adds.
 - Routing on-device: agreement = DVE mul + add-tree; softmax via ACT exp;
   sum over p via 0/1-masked (Delta) matmuls accumulating in PSUM;
   AllReduce (fp32) after iters 0 and 1 (iter-0 sum split to overlap the
   phase-1 tail).  Final iteration's partials summed + squashed on host.
"""

import numpy as np
import ml_dtypes

import concourse.bass as bass
import concourse.tile as tile
from concourse import bacc, mybir
from concourse.bass_utils import run_bass_kernel_spmd

F32 = mybir.dt.float32
BF16 = mybir.dt.bfloat16
AX = mybir.AxisListType
AF = mybir.ActivationFunctionType

B = 16      # batch
J = 32      # output capsules
D = 64      # output capsule dim
E = 64      # input capsule dim
JD = J * D  # 2048


def build_program(n_cores: int, n_groups: int):
    """Build the SPMD Bass program. Per core: P_loc = 8*n_groups capsules."""
    G = n_groups
    ploc = 8 * G
    npair = ploc // 2
    nblk = npair // 2          # DMA blocks of 2 pairs
    UFREE = G * JD             # u free elements per partition
    GS0 = max(G - 4, 1)        # s0 split point (early AR over g < GS0)

    nc = bacc.Bacc("TRN2", target_bir_lowering=False, debug=False,
                   num_devices=n_cores)

    w_dram = nc.dram_tensor("w", [nblk, 128, 2 * JD], BF16, kind="ExternalInput")
    x_dram = nc.dram_tensor("x", [128, npair * 32], BF16, kind="ExternalInput")
    out_dram = nc.dram_tensor("out", [16, JD], F32, kind="ExternalOutput")

    with tile.TileContext(nc) as tc:
        with (
            tc.tile_pool(name="const", bufs=1) as constp,
            tc.tile_pool(name="wpool", bufs=2) as wpool,
            tc.tile_pool(name="upool", bufs=1) as upool,
            tc.tile_pool(name="work", bufs=2) as work,
            tc.tile_pool(name="small", bufs=1) as small,
            tc.tile_pool(name="pmain", bufs=2, space="PSUM") as pmain,
            tc.tile_pool(name="pacc", bufs=1, space="PSUM") as pacc,
            tc.tile_pool(name="dram", bufs=1, space="DRAM") as dramp,
        ):
            # ---- static inputs -> SBUF ----
            x_sb = constp.tile([128, npair * 32], BF16)
            nc.sync.dma_start(x_sb[:], x_dram[:])
            # 0/1 mask constants built on-device (no DMA dependency):
            # delta[q, m] = (q % 16 == m); eye[r, q] = (q % 16 == r)
            I32 = mybir.dt.int32
            delta_sb = constp.tile([128, 16], BF16)
            qi = constp.tile([128, 128], I32)
            mi = constp.tile([128, 128], I32)
            ei = constp.tile([128, 128], F32)
            nc.gpsimd.iota(qi[:, :16], pattern=[[0, 16]], base=0,
                           channel_multiplier=1)
            nc.vector.tensor_scalar(qi[:, :16], qi[:, :16], 15, None,
                                    op0=mybir.AluOpType.bitwise_and)
            nc.gpsimd.iota(mi[:, :16], pattern=[[1, 16]], base=0,
                           channel_multiplier=0)
            nc.vector.tensor_tensor(ei[:, :16], qi[:, :16], mi[:, :16],
                                    op=mybir.AluOpType.is_equal)
            nc.vector.tensor_copy(delta_sb[:], ei[:, :16])
            eye_sb = constp.tile([16, 128], BF16)
            nc.gpsimd.iota(qi[:16, :], pattern=[[1, 128]], base=0,
                           channel_multiplier=0)
            nc.vector.tensor_scalar(qi[:16, :], qi[:16, :], 15, None,
                                    op0=mybir.AluOpType.bitwise_and)
            nc.gpsimd.iota(mi[:16, :], pattern=[[0, 128]], base=0,
                           channel_multiplier=1)
            nc.vector.tensor_tensor(ei[:16, :], qi[:16, :], mi[:16, :],
                                    op=mybir.AluOpType.is_equal)
            nc.vector.tensor_copy(eye_sb[:], ei[:16, :])

            u_sb = upool.tile([128, UFREE], BF16)

            # ---- collective helpers ----
            n_cc = [0]

            def all_reduce(src_f32, nelem=JD):
                i = n_cc[0]
                n_cc[0] += 1
                cin = dramp.tile([16, nelem], F32, tag=f"cin{i}", name=f"cc_in{i}")
                cout = dramp.tile([16, nelem], F32, tag=f"cout{i}",
                                  addr_space="Shared" if n_cores > 4 else "Local",
                                  name=f"cc_out{i}")
                nc.gpsimd.dma_start(cin[:], src_f32[:])
                nc.gpsimd.collective_compute(
                    "AllReduce", mybir.AluOpType.add,
                    replica_groups=[list(range(n_cores))],
                    ins=[cin.opt()], outs=[cout.opt()],
                )
                tag = "ar_a" if i % 2 == 0 else "ar_b"
                dst = small.tile([16, nelem], F32, tag=tag, name=f"ar_dst{i}")
                nc.gpsimd.dma_start(dst[:], cout[:])
                return dst

            # warmup collective to absorb first-CC setup cost (overlaps phase 1)
            warm = small.tile([16, 16], F32, tag="warm", name="warm")
            nc.vector.memset(warm[:], 0.0)
            all_reduce(warm, nelem=16)

            # ---- phase 1: stream W, matmul u, evict, accumulate s0 ----
            # s0 accumulated in two psum groups: g < GS0 (early AR) and rest.
            s0_parts = []
            ps0 = pacc.tile([16, JD], F32, tag="pacc", name="ps0_a")
            for g in range(G):
                if g == GS0:
                    s0a = small.tile([16, JD], F32, tag="s_loc", name="s0a")
                    nc.scalar.copy(s0a[:], ps0[:])
                    s0_parts.append(all_reduce(s0a))
                    ps0 = pacc.tile([16, JD], F32, tag="pacc", name="ps0_b")
                first, last = (g == 0 or g == GS0), (g == GS0 - 1 or g == G - 1)
                wtiles = []
                for half in range(2):
                    wt = wpool.tile([128, 2 * JD], BF16, tag="w",
                                    name=f"wt{g}_{half}", bufs=4)
                    eng = nc.sync if (2 * g + half) % 2 == 0 else nc.scalar
                    eng.dma_start(wt[:], w_dram[2 * g + half])
                    wtiles.append(wt)
                for hn in range(2):
                    pm = pmain.tile([128, 1024], F32, tag="pmain",
                                    name=f"pm{g}_{hn}")
                    for ns in range(2):
                        for cg in range(4):
                            pi = 4 * g + cg
                            lhsT = x_sb[:, pi * 32:(pi + 1) * 32]
                            half, cgl = divmod(cg, 2)
                            base = cgl * JD + hn * 1024
                            nc.tensor.matmul(
                                pm[32 * cg:32 * cg + 32, ns * 512:(ns + 1) * 512],
                                lhsT,
                                wtiles[half][:, base + ns * 512:
                                             base + (ns + 1) * 512],
                                tile_position=(0, 32 * cg),
                            )
                    off = g * JD + hn * 1024
                    nc.scalar.copy(u_sb[:, off:off + 1024], pm[:])
                    for ns in range(2):
                        nc.tensor.matmul(
                            ps0[:, hn * 1024 + ns * 512: hn * 1024 + (ns + 1) * 512],
                            delta_sb[:],
                            u_sb[:, off + ns * 512: off + (ns + 1) * 512],
                            start=first, stop=last,
                            skip_group_check=True,
                        )

            s0b = small.tile([16, JD], F32, tag="s_loc", name="s0b")
            nc.scalar.copy(s0b[:], ps0[:])
            s0_parts.append(all_reduce(s0b))

            # ---- squash + broadcast v to all 128 partitions (bf16) ----
            # s layout is [16, (d, j)]
            v_sb = constp.tile([128, JD], BF16)

            def squash_broadcast(s_sb, scale):
                # v = s*scale * sqrt(T)/(1+T), T = scale^2 * sum_d s^2
                # = s * [scale^2*sqrt(t_raw) * recip(1 + scale^2*t_raw)]
                s2t = small.tile([16, JD], F32, tag="s_loc", name="s2t")
                nc.vector.tensor_mul(s2t[:], s_sb[:], s_sb[:])
                t = small.tile([16, J], F32, tag="t", name="t")
                nc.vector.reduce_sum(t[:], s2t[:].rearrange("p (d j) -> p j d", d=D),
                                     axis=AX.X)
                st = small.tile([16, J], F32, tag="st", name="st")
                nc.scalar.sqrt(st[:], t[:])
                den = small.tile([16, J], F32, tag="den", name="den")
                nc.vector.tensor_scalar(den[:], t[:], scale * scale, 1.0,
                                        op0=mybir.AluOpType.mult,
                                        op1=mybir.AluOpType.add)
                rec = small.tile([16, J], F32, tag="rec", name="rec")
                nc.vector.reciprocal(rec[:], den[:])
                f = small.tile([16, J], F32, tag="f", name="f")
                nc.vector.scalar_tensor_tensor(f[:], st[:], scale * scale, rec[:],
                                               op0=mybir.AluOpType.mult,
                                               op1=mybir.AluOpType.mult)
                v16 = small.tile([16, JD], BF16, tag="v16", name="v16")
                nc.vector.tensor_mul(
                    v16[:].rearrange("p (d j) -> p d j", d=D),
                    s_sb[:].rearrange("p (d j) -> p d j", d=D),
                    f[:].unsqueeze(1).broadcast_to([16, D, J]),
                )
                for hn in range(2):
                    pv = pmain.tile([128, 1024], F32, tag="pmain", name=f"pv{hn}")
                    for ns in range(2):
                        nc.tensor.matmul(
                            pv[:, ns * 512:(ns + 1) * 512], eye_sb[:],
                            v16[:, hn * 1024 + ns * 512: hn * 1024 + (ns + 1) * 512])
                    nc.scalar.copy(v_sb[:, hn * 1024:(hn + 1) * 1024], pv[:])

            if len(s0_parts) == 2:
                s0 = s0_parts[1]
                nc.vector.tensor_add(s0[:], s0[:], s0_parts[0][:])
            else:
                s0 = s0_parts[0]
            squash_broadcast(s0, 1.0 / J)

            # ---- routing iterations ----
            bij = constp.tile([128, G * J], F32)
            nc.vector.memset(bij[:], 0.0)
            a_sb = constp.tile([128, G * J], F32)

            GC = 4 if G % 4 == 0 else 1   # groups per chunk
            NCH = G // GC
            CH = GC * JD                   # u elems per chunk per partition

            for it in (1, 2):
                ps = pacc.tile([16, JD], F32, tag="pacc", name=f"ps_it{it}")
                for h in range(NCH):
                    u_ch = u_sb[:, h * CH:(h + 1) * CH]
                    u3 = u_ch.rearrange("p (g q) -> p g q", g=GC)
                    # agreement: tmp = u * v (v broadcast over g; 2x mode)
                    tmp = work.tile([128, CH], BF16, tag="tmp", name="tmp")
                    nc.vector.tensor_mul(
                        tmp[:].rearrange("p (g q) -> p g q", g=GC),
                        u3,
                        v_sb[:].unsqueeze(1).broadcast_to([128, GC, JD]),
                    )
                    # pairwise add-tree over d: 64->32->16->8->4->2->1
                    cur, dl = tmp, D
                    while dl > 2:
                        nxt = work.tile([128, GC * (dl // 2) * J], BF16,
                                        tag=f"tr{dl}", name=f"tr{dl}",
                                        bufs=1 if dl >= 32 else 2)
                        c4 = cur[:].rearrange("p (g d j) -> p g d j", g=GC, d=dl)
                        nc.vector.tensor_add(
                            nxt[:].rearrange("p (g d j) -> p g d j", g=GC, d=dl // 2),
                            c4[:, :, 0:dl // 2, :], c4[:, :, dl // 2:dl, :])
                        cur, dl = nxt, dl // 2
                    # last level -> fp32 a, accumulated into bij
                    a_h = a_sb[:, h * GC * J:(h + 1) * GC * J]
                    c4 = cur[:].rearrange("p (g d j) -> p g d j", g=GC, d=2)
                    nc.vector.tensor_add(
                        a_h.rearrange("p (g j) -> p g j", g=GC).unsqueeze(2),
                        c4[:, :, 0:1, :], c4[:, :, 1:2, :])
                    b_h = bij[:, h * GC * J:(h + 1) * GC * J]
                    nc.vector.tensor_add(b_h, b_h, a_h)
                # batched softmax over j for all groups (one ACT exp)
                b3 = bij[:].rearrange("p (g j) -> p g j", g=G)
                mx = small.tile([128, G], F32, tag="mx", name="mx", bufs=2)
                nc.vector.reduce_max(mx[:], b3, axis=AX.X)
                eh = small.tile([128, G * J], F32, tag="eh", name="eh", bufs=2)
                eh3 = eh[:].rearrange("p (g j) -> p g j", g=G)
                nc.vector.tensor_sub(eh3, b3,
                                     mx[:].unsqueeze(2).broadcast_to([128, G, J]))
                nc.scalar.activation(eh[:], eh[:], AF.Exp)
                se = small.tile([128, G], F32, tag="se", name="se", bufs=2)
                nc.vector.reduce_sum(se[:], eh3, axis=AX.X)
                re = small.tile([128, G], F32, tag="re", name="re", bufs=2)
                nc.vector.reciprocal(re[:], se[:])
                c_full = small.tile([128, G * J], BF16, tag="c_h", name="c_full",
                                    bufs=2)
                nc.vector.tensor_mul(
                    c_full[:].rearrange("p (g j) -> p g j", g=G), eh3,
                    re[:].unsqueeze(2).broadcast_to([128, G, J]))
                for h in range(NCH):
                    u_ch = u_sb[:, h * CH:(h + 1) * CH]
                    u4 = u_ch.rearrange("p (g d j) -> p g d j", g=GC, d=D)
                    # cu = u * c (c broadcast over middle d; 2x mode)
                    cu = work.tile([128, CH], BF16, tag="tmp", name="cu")
                    nc.vector.tensor_mul(
                        cu[:].rearrange("p (g d j) -> p g d j", g=GC, d=D),
                        u4,
                        c_full[:, h * GC * J:(h + 1) * GC * J]
                            .rearrange("p (g j) -> p g j", g=GC)
                            .unsqueeze(2).broadcast_to([128, GC, D, J]),
                    )
                    # s += sum_k cu  (Delta matmuls, accumulate over chunks)
                    for gg in range(GC):
                        for ns in range(4):
                            nc.tensor.matmul(
                                ps[:, ns * 512:(ns + 1) * 512],
                                delta_sb[:],
                                cu[:, gg * JD + ns * 512: gg * JD + (ns + 1) * 512],
                                start=(h == 0 and gg == 0),
                                stop=(h == NCH - 1 and gg == GC - 1),
                                skip_group_check=True,
                            )
                if it == 1:
                    s_loc = small.tile([16, JD], F32, tag="s_loc", name="s_loc")
                    nc.scalar.copy(s_loc[:], ps[:])
                    s1 = all_reduce(s_loc)
                    squash_broadcast(s1, 1.0)
                else:
                    s2_sb = small.tile([16, JD], F32, tag="s_loc", name="s2_sb")
                    nc.scalar.copy(s2_sb[:], ps[:])
                    nc.sync.dma_start(out_dram[:], s2_sb[:])

    nc.compile()
    return nc


def pack_inputs(inp, W, b, n_cores: int, n_groups: int):
    """Host-side packing -> per-core in_maps. W columns in (d, j) order."""
    P = inp.shape[1]
    G = n_groups
    ploc = 8 * G
    npair = ploc // 2
    nblk = npair // 2
    assert n_cores * ploc == P

    bf = ml_dtypes.bfloat16
    if b is not None and np.any(b):
        raise NotImplementedError("nonzero bias b is not supported")
    # W[0]: [P, J, E, D] -> [P, E, (D, J)]
    Wt = np.ascontiguousarray(W[0].transpose(0, 2, 3, 1)).reshape(P, E, JD)
    Wp = Wt.reshape(P // 2, 2 * E, JD)
    Wb = Wp.reshape(n_cores, nblk, 2, 2 * E, JD).transpose(0, 1, 3, 2, 4)
    w_dev = np.ascontiguousarray(Wb).reshape(n_cores, nblk, 128, 2 * JD).astype(bf)

    # x: [B, P, E] -> block diag lhsT [c, 128, npair*32]
    inpT = inp.transpose(1, 2, 0)          # [P, E, B]
    arr = inpT.reshape(n_cores, npair, 2, E, B)
    x_dev = np.zeros((n_cores, 2, E, npair, 2, 16), np.float32)
    x_dev[:, 0, :, :, 0, :] = arr[:, :, 0].transpose(0, 2, 1, 3)
    x_dev[:, 1, :, :, 1, :] = arr[:, :, 1].transpose(0, 2, 1, 3)
    x_dev = x_dev.reshape(n_cores, 128, npair * 32).astype(bf)

    in_maps = []
    for c in range(n_cores):
        in_maps.append({"w": w_dev[c], "x": x_dev[c]})
    return in_maps


def squash_np(x):
    s2 = np.sum(x * x, axis=-1, keepdims=True)
    return x * (s2 / (1.0 + s2)) / np.sqrt(s2)


_CACHE = {}


def kernel(inp: np.ndarray, W: np.ndarray, b: np.ndarray) -> np.ndarray:
    n_cores, n_groups = 8, 16
    inp = np.asarray(inp, dtype=np.float32)
    W = np.asarray(W, dtype=np.float32)
    b = np.asarray(b, dtype=np.float32)

    key = (n_cores, n_groups)
    if key not in _CACHE:
        _CACHE[key] = build_program(n_cores, n_groups)
    nc = _CACHE[key]

    in_maps = pack_inputs(inp, W, b, n_cores, n_groups)
    res = run_bass_kernel_spmd(nc, in_maps, core_ids=list(range(n_cores)))
    s2 = np.zeros((16, JD), np.float64)
    for r in res.results:
        s2 += r["out"].astype(np.float64)
    # s layout [16, (d, j)] -> [B, J, D]
    v = squash_np(s2.reshape(B, D, J).transpose(0, 2, 1))
    return v.astype(np.float32)



# revision 1
# speedup vs baseline: 1.1087x; 1.1087x over previous
"""Trainium2 Bass kernel for CapsuleLayer dynamic routing.

Problem: u = einsum('bpe,pjed->bpjd', inp, W[0]) + b, then 3 routing
iterations (softmax over j, weighted sum over p, squash) -> vj [B,J,D].

Shapes: B=16, P=1024, J=32, Dp=D=64.  W is 512MB fp32 -> DMA dominated.

Strategy (8 NeuronCores):
 - Shard P across cores: 128 p's per core; all batches on every core.
 - Host packs W (bf16) as stacked pairs: rhs[pair] = [W_pe; W_po] with
   K=128=(2 p's x 64 e).  lhsT[pair] = block-diag([inp_pe, inp_po]) so one
   matmul computes u for 2 capsules with full contraction rows, M=32.
 - 4 pairs per PSUM round via col-group tile_position -> PSUM [128,1024]
   halves, double buffered -> wide evictions (cast bf16) into SBUF
   u[(k,b), (g,d,j)].  Free order (d,j) keeps every routing DVE op in the
   bf16 2x mode: v/c broadcasts are outer/middle dims and the d-reduction
   is a pairwise tree of step-1 adds.
 - Routing on-device: agreement = DVE mul + add-tree; softmax via ACT exp;
   sum over p via 0/1-masked (Delta) matmuls accumulating in PSUM;
   AllReduce (fp32) after iters 0 and 1 (iter-0 sum split to overlap the
   phase-1 tail).  Final iteration's partials summed + squashed on host.
"""

import numpy as np
import ml_dtypes

import concourse.bass as bass
import concourse.tile as tile
from concourse import bacc, mybir
from concourse.bass_utils import run_bass_kernel_spmd

F32 = mybir.dt.float32
BF16 = mybir.dt.bfloat16
AX = mybir.AxisListType
AF = mybir.ActivationFunctionType

B = 16      # batch
J = 32      # output capsules
D = 64      # output capsule dim
E = 64      # input capsule dim
JD = J * D  # 2048


def build_program(n_cores: int, n_groups: int):
    """Build the SPMD Bass program. Per core: P_loc = 8*n_groups capsules."""
    G = n_groups
    ploc = 8 * G
    npair = ploc // 2
    nblk = npair // 2          # DMA blocks of 2 pairs
    UFREE = G * JD             # u free elements per partition
    GS0 = max(G - 4, 1)        # s0 split point (early AR over g < GS0)

    nc = bacc.Bacc("TRN2", target_bir_lowering=False, debug=False,
                   num_devices=n_cores)

    w_dram = nc.dram_tensor("w", [nblk, 128, 2 * JD], BF16, kind="ExternalInput")
    x_dram = nc.dram_tensor("x", [128, npair * 32], BF16, kind="ExternalInput")
    out_dram = nc.dram_tensor("out", [16, JD], F32, kind="ExternalOutput")

    with tile.TileContext(nc) as tc:
        with (
            tc.tile_pool(name="const", bufs=1) as constp,
            tc.tile_pool(name="wpool", bufs=2) as wpool,
            tc.tile_pool(name="upool", bufs=1) as upool,
            tc.tile_pool(name="work", bufs=2) as work,
            tc.tile_pool(name="small", bufs=1) as small,
            tc.tile_pool(name="pmain", bufs=2, space="PSUM") as pmain,
            tc.tile_pool(name="pacc", bufs=1, space="PSUM") as pacc,
            tc.tile_pool(name="dram", bufs=1, space="DRAM") as dramp,
        ):
            # ---- static inputs -> SBUF ----
            x_sb = constp.tile([128, npair * 32], BF16)
            nc.sync.dma_start(x_sb[:], x_dram[:])
            # 0/1 mask constants built on-device (no DMA dependency):
            # delta[q, m] = (q % 16 == m); eye[r, q] = (q % 16 == r)
            I32 = mybir.dt.int32
            delta_sb = constp.tile([128, 16], BF16)
            qi = constp.tile([128, 128], I32)
            mi = constp.tile([128, 128], I32)
            ei = constp.tile([128, 128], F32)
            nc.gpsimd.iota(qi[:, :16], pattern=[[0, 16]], base=0,
                           channel_multiplier=1)
            nc.vector.tensor_scalar(qi[:, :16], qi[:, :16], 15, None,
                                    op0=mybir.AluOpType.bitwise_and)
            nc.gpsimd.iota(mi[:, :16], pattern=[[1, 16]], base=0,
                           channel_multiplier=0)
            nc.vector.tensor_tensor(ei[:, :16], qi[:, :16], mi[:, :16],
                                    op=mybir.AluOpType.is_equal)
            nc.vector.tensor_copy(delta_sb[:], ei[:, :16])
            eye_sb = constp.tile([16, 128], BF16)
            nc.gpsimd.iota(qi[:16, :], pattern=[[1, 128]], base=0,
                           channel_multiplier=0)
            nc.vector.tensor_scalar(qi[:16, :], qi[:16, :], 15, None,
                                    op0=mybir.AluOpType.bitwise_and)
            nc.gpsimd.iota(mi[:16, :], pattern=[[0, 128]], base=0,
                           channel_multiplier=1)
            nc.vector.tensor_tensor(ei[:16, :], qi[:16, :], mi[:16, :],
                                    op=mybir.AluOpType.is_equal)
            nc.vector.tensor_copy(eye_sb[:], ei[:16, :])

            u_sb = upool.tile([128, UFREE], BF16)

            # ---- collective helpers ----
            n_cc = [0]

            def all_reduce(src_f32, nelem=JD):
                i = n_cc[0]
                n_cc[0] += 1
                cin = dramp.tile([16, nelem], F32, tag=f"cin{i}", name=f"cc_in{i}")
                cout = dramp.tile([16, nelem], F32, tag=f"cout{i}",
                                  addr_space="Shared" if n_cores > 4 else "Local",
                                  name=f"cc_out{i}")
                nc.gpsimd.dma_start(cin[:], src_f32[:])
                nc.gpsimd.collective_compute(
                    "AllReduce", mybir.AluOpType.add,
                    replica_groups=[list(range(n_cores))],
                    ins=[cin.opt()], outs=[cout.opt()],
                )
                tag = "ar_a" if i % 2 == 0 else "ar_b"
                dst = small.tile([16, nelem], F32, tag=tag, name=f"ar_dst{i}")
                nc.gpsimd.dma_start(dst[:], cout[:])
                return dst

            # warmup collective to absorb first-CC setup cost (overlaps phase 1)
            warm = small.tile([16, 16], F32, tag="warm", name="warm")
            nc.vector.memset(warm[:], 0.0)
            all_reduce(warm, nelem=16)

            # ---- phase 1: stream W, matmul u, evict, accumulate s0 ----
            # s0 accumulated in two psum groups: g < GS0 (early AR) and rest.
            s0_parts = []
            ps0 = pacc.tile([16, JD], F32, tag="pacc", name="ps0_a")
            for g in range(G):
                if g == GS0:
                    s0a = small.tile([16, JD], F32, tag="s_loc", name="s0a")
                    nc.scalar.copy(s0a[:], ps0[:])
                    s0_parts.append(all_reduce(s0a))
                    ps0 = pacc.tile([16, JD], F32, tag="pacc", name="ps0_b")
                first, last = (g == 0 or g == GS0), (g == GS0 - 1 or g == G - 1)
                wtiles = []
                for half in range(2):
                    wt = wpool.tile([128, 2 * JD], BF16, tag="w",
                                    name=f"wt{g}_{half}", bufs=4)
                    eng = nc.sync if (2 * g + half) % 2 == 0 else nc.scalar
                    eng.dma_start(wt[:], w_dram[2 * g + half])
                    wtiles.append(wt)
                for hn in range(2):
                    pm = pmain.tile([128, 1024], F32, tag="pmain",
                                    name=f"pm{g}_{hn}")
                    for ns in range(2):
                        for cg in range(4):
                            pi = 4 * g + cg
                            lhsT = x_sb[:, pi * 32:(pi + 1) * 32]
                            half, cgl = divmod(cg, 2)
                            base = cgl * JD + hn * 1024
                            nc.tensor.matmul(
                                pm[32 * cg:32 * cg + 32, ns * 512:(ns + 1) * 512],
                                lhsT,
                                wtiles[half][:, base + ns * 512:
                                             base + (ns + 1) * 512],
                                tile_position=(0, 32 * cg),
                            )
                    off = g * JD + hn * 1024
                    nc.scalar.copy(u_sb[:, off:off + 1024], pm[:])
                    for ns in range(2):
                        nc.tensor.matmul(
                            ps0[:, hn * 1024 + ns * 512: hn * 1024 + (ns + 1) * 512],
                            delta_sb[:],
                            u_sb[:, off + ns * 512: off + (ns + 1) * 512],
                            start=first, stop=last,
                            skip_group_check=True,
                        )

            s0b = small.tile([16, JD], F32, tag="s_loc", name="s0b")
            nc.scalar.copy(s0b[:], ps0[:])
            s0_parts.append(all_reduce(s0b))

            # ---- squash + broadcast v to all 128 partitions (bf16) ----
            # s layout is [16, (d, j)]
            v_sb = constp.tile([128, JD], BF16)

            def squash_broadcast(s_sb, scale):
                # v = s*scale * sqrt(T)/(1+T), T = scale^2 * sum_d s^2
                # = s * [scale^2*sqrt(t_raw) * recip(1 + scale^2*t_raw)]
                s2t = small.tile([16, JD], F32, tag="s_loc", name="s2t")
                nc.vector.tensor_mul(s2t[:], s_sb[:], s_sb[:])
                t = small.tile([16, J], F32, tag="t", name="t")
                nc.vector.reduce_sum(t[:], s2t[:].rearrange("p (d j) -> p j d", d=D),
                                     axis=AX.X)
                st = small.tile([16, J], F32, tag="st", name="st")
                nc.scalar.sqrt(st[:], t[:])
                den = small.tile([16, J], F32, tag="den", name="den")
                nc.vector.tensor_scalar(den[:], t[:], scale * scale, 1.0,
                                        op0=mybir.AluOpType.mult,
                                        op1=mybir.AluOpType.add)
                rec = small.tile([16, J], F32, tag="rec", name="rec")
                nc.vector.reciprocal(rec[:], den[:])
                f = small.tile([16, J], F32, tag="f", name="f")
                nc.vector.scalar_tensor_tensor(f[:], st[:], scale * scale, rec[:],
                                               op0=mybir.AluOpType.mult,
                                               op1=mybir.AluOpType.mult)
                v16 = small.tile([16, JD], BF16, tag="v16", name="v16")
                nc.vector.tensor_mul(
                    v16[:].rearrange("p (d j) -> p d j", d=D),
                    s_sb[:].rearrange("p (d j) -> p d j", d=D),
                    f[:].unsqueeze(1).broadcast_to([16, D, J]),
                )
                for hn in range(2):
                    pv = pmain.tile([128, 1024], F32, tag="pmain", name=f"pv{hn}")
                    for ns in range(2):
                        nc.tensor.matmul(
                            pv[:, ns * 512:(ns + 1) * 512], eye_sb[:],
                            v16[:, hn * 1024 + ns * 512: hn * 1024 + (ns + 1) * 512])
                    nc.scalar.copy(v_sb[:, hn * 1024:(hn + 1) * 1024], pv[:])

            if len(s0_parts) == 2:
                s0 = s0_parts[1]
                nc.vector.tensor_add(s0[:], s0[:], s0_parts[0][:])
            else:
                s0 = s0_parts[0]
            squash_broadcast(s0, 1.0 / J)

            # ---- routing iterations ----
            bij = constp.tile([128, G * J], F32)
            nc.vector.memset(bij[:], 0.0)
            a_sb = constp.tile([128, G * J], F32)

            GC = 4 if G % 4 == 0 else 1   # groups per chunk
            NCH = G // GC
            CH = GC * JD                   # u elems per chunk per partition

            for it in (1, 2):
                ps = pacc.tile([16, JD], F32, tag="pacc", name=f"ps_it{it}")
                for h in range(NCH):
                    u_ch = u_sb[:, h * CH:(h + 1) * CH]
                    u3 = u_ch.rearrange("p (g q) -> p g q", g=GC)
                    # agreement: tmp = u * v (v broadcast over g; 2x mode)
                    tmp = work.tile([128, CH], BF16, tag="tmp", name="tmp")
                    nc.vector.tensor_mul(
                        tmp[:].rearrange("p (g q) -> p g q", g=GC),
                        u3,
                        v_sb[:].unsqueeze(1).broadcast_to([128, GC, JD]),
                    )
                    # pairwise add-tree over d: 64->32->16->8->4->2->1
                    cur, dl = tmp, D
                    while dl > 2:
                        nxt = work.tile([128, GC * (dl // 2) * J], BF16,
                                        tag=f"tr{dl}", name=f"tr{dl}",
                                        bufs=1 if dl >= 32 else 2)
                        c4 = cur[:].rearrange("p (g d j) -> p g d j", g=GC, d=dl)
                        nc.vector.tensor_add(
                            nxt[:].rearrange("p (g d j) -> p g d j", g=GC, d=dl // 2),
                            c4[:, :, 0:dl // 2, :], c4[:, :, dl // 2:dl, :])
                        cur, dl = nxt, dl // 2
                    # last level -> fp32 a, accumulated into bij
                    a_h = a_sb[:, h * GC * J:(h + 1) * GC * J]
                    c4 = cur[:].rearrange("p (g d j) -> p g d j", g=GC, d=2)
                    nc.vector.tensor_add(
                        a_h.rearrange("p (g j) -> p g j", g=GC).unsqueeze(2),
                        c4[:, :, 0:1, :], c4[:, :, 1:2, :])
                    b_h = bij[:, h * GC * J:(h + 1) * GC * J]
                    nc.vector.tensor_add(b_h, b_h, a_h)
                # batched softmax over j for all groups (one ACT exp)
                b3 = bij[:].rearrange("p (g j) -> p g j", g=G)
                mx = small.tile([128, G], F32, tag="mx", name="mx", bufs=2)
                nc.vector.reduce_max(mx[:], b3, axis=AX.X)
                eh = small.tile([128, G * J], F32, tag="eh", name="eh", bufs=2)
                eh3 = eh[:].rearrange("p (g j) -> p g j", g=G)
                nc.vector.tensor_sub(eh3, b3,
                                     mx[:].unsqueeze(2).broadcast_to([128, G, J]))
                nc.scalar.activation(eh[:], eh[:], AF.Exp)
                se = small.tile([128, G], F32, tag="se", name="se", bufs=2)
                nc.vector.reduce_sum(se[:], eh3, axis=AX.X)
                re = small.tile([128, G], F32, tag="re", name="re", bufs=2)
                nc.vector.reciprocal(re[:], se[:])
                c_full = small.tile([128, G * J], BF16, tag="c_h", name="c_full",
                                    bufs=2)
                nc.vector.tensor_mul(
                    c_full[:].rearrange("p (g j) -> p g j", g=G), eh3,
                    re[:].unsqueeze(2).broadcast_to([128, G, J]))
                for h in range(NCH):
                    u_ch = u_sb[:, h * CH:(h + 1) * CH]
                    u4 = u_ch.rearrange("p (g d j) -> p g d j", g=GC, d=D)
                    # cu = u * c (c broadcast over middle d; 2x mode)
                    cu = work.tile([128, CH], BF16, tag="tmp", name="cu")
                    nc.vector.tensor_mul(
                        cu[:].rearrange("p (g d j) -> p g d j", g=GC, d=D),
                        u4,
                        c_full[:, h * GC * J:(h + 1) * GC * J]
                            .rearrange("p (g j) -> p g j", g=GC)
                            .unsqueeze(2).broadcast_to([128, GC, D, J]),
                    )
                    # s += sum_k cu  (Delta matmuls, accumulate over chunks)
                    for gg in range(GC):
                        for ns in range(4):
                            nc.tensor.matmul(
                                ps[:, ns * 512:(ns + 1) * 512],
                                delta_sb[:],
                                cu[:, gg * JD + ns * 512: gg * JD + (ns + 1) * 512],
                                start=(h == 0 and gg == 0),
                                stop=(h == NCH - 1 and gg == GC - 1),
                                skip_group_check=True,
                            )
                if it == 1:
                    s_loc = small.tile([16, JD], F32, tag="s_loc", name="s_loc")
                    nc.scalar.copy(s_loc[:], ps[:])
                    s1 = all_reduce(s_loc)
                    squash_broadcast(s1, 1.0)
                else:
                    s2_sb = small.tile([16, JD], F32, tag="s_loc", name="s2_sb")
                    nc.scalar.copy(s2_sb[:], ps[:])
                    nc.sync.dma_start(out_dram[:], s2_sb[:])

    nc.compile()
    return nc


def pack_inputs(inp, W, b, n_cores: int, n_groups: int):
    """Host-side packing -> per-core in_maps. W columns in (d, j) order."""
    P = inp.shape[1]
    G = n_groups
    ploc = 8 * G
    npair = ploc // 2
    nblk = npair // 2
    assert n_cores * ploc == P

    bf = ml_dtypes.bfloat16
    if b is not None and np.any(b):
        raise NotImplementedError("nonzero bias b is not supported")
    # W[0]: [P, J, E, D] -> [P, E, (D, J)]
    Wt = np.ascontiguousarray(W[0].transpose(0, 2, 3, 1)).reshape(P, E, JD)
    Wp = Wt.reshape(P // 2, 2 * E, JD)
    Wb = Wp.reshape(n_cores, nblk, 2, 2 * E, JD).transpose(0, 1, 3, 2, 4)
    w_dev = np.ascontiguousarray(Wb).reshape(n_cores, nblk, 128, 2 * JD).astype(bf)

    # x: [B, P, E] -> block diag lhsT [c, 128, npair*32]
    inpT = inp.transpose(1, 2, 0)          # [P, E, B]
    arr = inpT.reshape(n_cores, npair, 2, E, B)
    x_dev = np.zeros((n_cores, 2, E, npair, 2, 16), np.float32)
    x_dev[:, 0, :, :, 0, :] = arr[:, :, 0].transpose(0, 2, 1, 3)
    x_dev[:, 1, :, :, 1, :] = arr[:, :, 1].transpose(0, 2, 1, 3)
    x_dev = x_dev.reshape(n_cores, 128, npair * 32).astype(bf)

    in_maps = []
    for c in range(n_cores):
        in_maps.append({"w": w_dev[c], "x": x_dev[c]})
    return in_maps


def squash_np(x):
    s2 = np.sum(x * x, axis=-1, keepdims=True)
    return x * (s2 / (1.0 + s2)) / np.sqrt(s2)


_CACHE = {}


def kernel(inp: np.ndarray, W: np.ndarray, b: np.ndarray) -> np.ndarray:
    n_cores, n_groups = 8, 16
    inp = np.asarray(inp, dtype=np.float32)
    W = np.asarray(W, dtype=np.float32)
    b = np.asarray(b, dtype=np.float32)

    key = (n_cores, n_groups)
    if key not in _CACHE:
        _CACHE[key] = build_program(n_cores, n_groups)
    nc = _CACHE[key]

    in_maps = pack_inputs(inp, W, b, n_cores, n_groups)
    res = run_bass_kernel_spmd(nc, in_maps, core_ids=list(range(n_cores)))
    s2 = np.zeros((16, JD), np.float64)
    for r in res.results:
        s2 += r["out"].astype(np.float64)
    # s layout [16, (d, j)] -> [B, J, D]
    v = squash_np(s2.reshape(B, D, J).transpose(0, 2, 1))
    return v.astype(np.float32)

